# revision 52
# baseline (speedup 1.0000x reference)
"""BiGNN message-passing kernel for Trainium2 (8 NeuronCores, Bass/Tile).

Reference computation (N=100000 nodes, E=600000 edges, D=128):
    msgs = vals[:, None] * features[cols]            # gather + scale
    x    = segment_sum(msgs, rows)                   # scatter-add to rows
    out  = (features + x) @ W1 + b1 + (x * features) @ W2 + b2

Sharding: destination nodes (rows) are sharded across the 8 cores, 12500
each; `features` is replicated into every core's HBM, so the per-edge
source gather is core-local (no collectives).

The critical path is GPSIMD descriptor generation for the per-edge
feature gather (dma_gather on 4 SWDGE queues, ~2ns/row serial on the
Pool engine), so the layout minimizes gathered rows and keeps everything
else off that path:

- Edges are packed per (tile-group, feature-chunk) section with NO
  per-tile alignment: each (tile, chunk) run is laid out contiguously,
  runs back to back, sections rounded up to 128 slots. One dma_gather
  per section (32 total vs 52 in the padded-block layout, ~15% fewer
  gathered rows than 128-aligned packing). The last two groups are
  small (8 and 6 tiles) so the post-last-gather epilogue tail is short.
- The segment-sum runs on TensorE as one matmul per (tile, 128-slot
  block) USE: xT[f,d] += G_blk[e,f].T @ S_use[e,d]. Because blocks can
  straddle tile boundaries, S carries one 128-wide one-hot column group
  PER USE (not per block): rows of foreign tiles are zero in that use's
  columns, so the matmul always contracts over the full 128 partitions
  (PE sub-partition operands crash the device, and on-device S builds
  measured ~3x slower on DVE than the ~34MB/core HWDGE stream of
  host-built S, which overlaps the gather-limited critical path).
- Dense epilogue in transposed [feature, node] layout:
  outT = W1.T @ (fT + xT) + W2.T @ (xT * fT) + (b1 + b2); the host
  transposes per-core outputs back.
"""

import numpy as np

P = 128
D = 128
N_NODES = 100000
N_EDGES = 600000
N_CORES = 8
NCHUNKS = 4  # feature-table column chunks (int16 gather index reach)
GROUP_SIZES = [14, 14, 14, 14, 14, 14, 8, 6]  # dest tiles per group; small
# tail groups shrink the post-last-gather epilogue on the critical path

_LAST_RESULTS = None  # BassKernelResults of the most recent run (for test.py)


def _prep(rows, cols, vals, n_nodes, n_cores):
    """Host-side edge reorganization into the shared slot/use schedule.

    Returns (sched, per_core):
      sched: tiles/npc/cc/NG/secblk/C/NB/TOT/NU/U0/tile_uses
      per_core[c]: idx16 [128, TOT/16] int16,
                   dest16/val16 [128, NU] fp16 (per-use masked columns)
    """
    npc = n_nodes // n_cores
    tiles = (npc + P - 1) // P
    cc = n_nodes // NCHUNKS
    assert sum(GROUP_SIZES) == tiles
    NG = len(GROUP_SIZES)
    gb = np.concatenate([[0], np.cumsum(GROUP_SIZES)])  # group tile bounds
    g_of = np.repeat(np.arange(NG), GROUP_SIZES)  # tile -> group
    assert n_nodes % NCHUNKS == 0

    rows = np.asarray(rows, dtype=np.int64)
    cols = np.asarray(cols, dtype=np.int64)
    vals = np.asarray(vals, dtype=np.float32)
    e = rows.shape[0]

    core = rows // npc
    local = rows - core * npc
    t_idx = local // P
    dest = (local - t_idx * P).astype(np.int64)
    j_idx = cols // cc
    lidx = cols - j_idx * cc

    key = (core * tiles + t_idx) * NCHUNKS + j_idx
    order = np.argsort(key, kind="stable")
    cnt = np.bincount(key, minlength=n_cores * tiles * NCHUNKS).reshape(
        n_cores, tiles, NCHUNKS
    )
    L = cnt.max(axis=0).astype(np.int64)  # [tiles, NCHUNKS] shared run lengths

    # sections (group, chunk): contiguous unaligned runs, rounded to 128
    A = np.zeros((tiles, NCHUNKS), dtype=np.int64)  # run offset within section
    secblk = np.zeros((NG, NCHUNKS), dtype=np.int64)
    for g in range(NG):
        t0, t1 = int(gb[g]), int(gb[g + 1])
        for j in range(NCHUNKS):
            off = 0
            for t in range(t0, t1):
                A[t, j] = off
                off += L[t, j]
            secblk[g, j] = max((off + P - 1) // P, 1)
    C = np.zeros((NG, NCHUNKS), dtype=np.int64)  # global block offset
    nb = 0
    for g in range(NG):
        for j in range(NCHUNKS):
            C[g, j] = nb
            nb += secblk[g, j]
    NB = nb
    TOT = NB * P

    # per-edge slot: section base + run offset + rank within (core, t, j)
    starts_flat = np.concatenate([[0], np.cumsum(cnt.reshape(-1))[:-1]])
    rank = np.empty(e, dtype=np.int64)
    rank[order] = np.arange(e) - np.repeat(starts_flat, cnt.reshape(-1))
    slot = C[g_of[t_idx], j_idx] * P + A[t_idx, j_idx] + rank

    # shared slot->tile ownership map (core independent)
    tileof = np.full(TOT, -1, dtype=np.int64)
    for t in range(tiles):
        g = int(g_of[t])
        for j in range(NCHUNKS):
            base = C[g, j] * P + A[t, j]
            tileof[base : base + L[t, j]] = t

    # uses: one matmul (and one S column group) per (tile, touched block)
    sec_uses = {}  # (g, j) -> list of (t, gblk)
    tile_uses = [[] for _ in range(tiles)]
    ul0 = np.zeros((tiles, NCHUNKS), dtype=np.int64)  # first use (in-section)
    blk0 = np.zeros((tiles, NCHUNKS), dtype=np.int64)  # first block of run
    for g in range(NG):
        t0, t1 = int(gb[g]), int(gb[g + 1])
        for j in range(NCHUNKS):
            ulist = []
            for t in range(t0, t1):
                a = A[t, j]
                b = a + L[t, j]
                if b == a:
                    continue
                ul0[t, j] = len(ulist)
                blk0[t, j] = a // P
                for bi in range(a // P, (b + P - 1) // P):
                    ulist.append((t, int(C[g, j]) + bi))
            sec_uses[(g, j)] = ulist
    # exact per-section use-column layout (no padding)
    U0 = np.zeros((NG, NCHUNKS), dtype=np.int64)  # section first column
    nu = 0
    for g in range(NG):
        for j in range(NCHUNKS):
            U0[g, j] = nu
            nu += len(sec_uses[(g, j)])
    NU = nu
    for g in range(NG):
        for j in range(NCHUNKS):
            for ul, (t, gblk) in enumerate(sec_uses[(g, j)]):
                tile_uses[t].append((j, gblk, ul))
    # reorder each tile's uses to chunk-major (j asc) for the psum chain
    for t in range(tiles):
        tile_uses[t].sort(key=lambda u: (u[0], u[1]))

    # per-edge use column (global)
    u_edge = (
        U0[g_of[t_idx], j_idx]
        + ul0[t_idx, j_idx]
        + (slot // P - (C[g_of[t_idx], j_idx] + blk0[t_idx, j_idx]))
    )

    per_core = []
    for c in range(n_cores):
        m = core == c
        s = slot[m]
        idx_flat = np.zeros(TOT, dtype=np.int16)
        idx_flat[s] = lidx[m].astype(np.int16)
        idx16 = np.tile(np.ascontiguousarray(idx_flat.reshape(-1, 16).T), (8, 1))
        # host-built per-use one-hot S: S16[p, u*128 + d] = val
        S16 = np.zeros((P, NU * P), dtype=np.float16)
        S16[s % P, u_edge[m] * P + dest[m]] = vals[m].astype(np.float16)
        per_core.append(
            {
                "idx16": np.ascontiguousarray(idx16),
                "S16": S16,
            }
        )

    sched = {
        "tiles": tiles,
        "npc": npc,
        "cc": cc,
        "NG": NG,
        "secblk": secblk,
        "C": C,
        "NB": NB,
        "TOT": TOT,
        "NU": NU,
        "U0": U0,
        "gb": gb,
        "tile_uses": tile_uses,
    }
    return sched, per_core


def _build_program(n_nodes, sched):
    import concourse.bacc as bacc
    import concourse.mybir as mybir
    import concourse.tile as tile

    f32 = mybir.dt.float32
    f16 = mybir.dt.float16
    i16 = mybir.dt.int16

    npc = sched["npc"]
    cc = sched["cc"]
    NG = sched["NG"]
    secblk = sched["secblk"]
    C = sched["C"]
    NB = sched["NB"]
    TOT = sched["TOT"]
    NU = sched["NU"]
    U0 = sched["U0"]
    gb = sched["gb"]
    tiles = sched["tiles"]
    tile_uses = sched["tile_uses"]

    nc = bacc.Bacc(num_swdge_queues=4)
    feat16 = [
        nc.dram_tensor(f"feat16_{j}", [cc, D], f16, kind="ExternalInput")
        for j in range(NCHUNKS)
    ]
    featT = nc.dram_tensor("featT", [D, npc], f32, kind="ExternalInput")
    w1 = nc.dram_tensor("W1", [D, D], f16, kind="ExternalInput")
    w2 = nc.dram_tensor("W2", [D, D], f16, kind="ExternalInput")
    bsum = nc.dram_tensor("bsum", [D, 1], f32, kind="ExternalInput")
    idx16 = nc.dram_tensor("idx16", [P, TOT // 16], i16, kind="ExternalInput")
    s16 = nc.dram_tensor("S16", [P, NU * P], f16, kind="ExternalInput")
    outT = nc.dram_tensor("outT", [D, npc], f32, kind="ExternalOutput")

    with tile.TileContext(nc) as tc:
        with (
            tc.tile_pool(name="const", bufs=1) as constp,
            tc.tile_pool(name="idxp", bufs=2) as idxp,
            tc.tile_pool(name="gpool", bufs=2) as gpool,
            tc.tile_pool(name="spool", bufs=2) as spool,
            tc.tile_pool(name="ftpool", bufs=2) as ftpool,
            tc.tile_pool(name="dense", bufs=3) as densep,
            tc.tile_pool(name="ostage", bufs=2) as ostagep,
            tc.tile_pool(name="psx", bufs=6, space="PSUM") as psx,
            tc.tile_pool(name="pso", bufs=2, space="PSUM") as pso,
        ):
            # consts ride the scalar (ACT) ring so the group-0 gather index
            # load is first in the sync HWDGE queue
            w1_t = constp.tile([P, P], f16)
            nc.scalar.dma_start(out=w1_t[:], in_=w1[:, :])
            w2_t = constp.tile([P, P], f16)
            nc.scalar.dma_start(out=w2_t[:], in_=w2[:, :])
            bias_t = constp.tile([P, 1], f32)
            nc.scalar.dma_start(out=bias_t[:], in_=bsum[:, :])

            for g in range(NG):
                t0g = int(gb[g])
                t1g = int(gb[g + 1])
                gw = min(t1g * P, npc) - t0g * P
                b0g = int(C[g, 0])
                b1g = int(C[g, NCHUNKS - 1] + secblk[g, NCHUNKS - 1])
                gnb = b1g - b0g
                u0g = int(U0[g, 0])
                u1g = int(U0[g + 1, 0]) if g + 1 < NG else NU
                gnu = u1g - u0g

                # per-group streams: gather indices first (gathers wait on it)
                ix = idxp.tile([P, gnb * 8], i16, tag="ix")
                nc.sync.dma_start(out=ix[:], in_=idx16[:, b0g * 8 : b1g * 8])
                # host-built per-use one-hot S for the whole group
                S = spool.tile([P, gnu, P], f16, tag="S")
                nc.sync.dma_start(out=S[:], in_=s16[:, u0g * P : (u0g + gnu) * P])
                fT = ftpool.tile([P, gw], f32, tag="fT")
                nc.scalar.dma_start(out=fT[:], in_=featT[:, t0g * P : t0g * P + gw])

                Gs = {}
                for j in range(NCHUNKS):
                    nblk = int(secblk[g, j])
                    di = int(C[g, j]) - b0g
                    G = gpool.tile([P, nblk, P], f16, tag=f"G{j}")
                    # two half-gathers per section: finer generation/drain
                    # interleave keeps the Q7 descriptor generator from
                    # stalling on ring space at group boundaries
                    h0 = nblk // 2 if nblk > 1 else nblk
                    for hi, (c0, c1) in enumerate(((0, h0), (h0, nblk))):
                        if c1 == c0:
                            continue
                        n_idx = (c1 - c0) * P
                        nc.gpsimd.dma_gather(
                            G[:, c0:c1, :],
                            feat16[j][:, :],
                            ix[:, (di + c0) * 8 : (di + c0) * 8 + n_idx // 16],
                            n_idx,
                            n_idx,
                            D,
                            single_packet=False,
                            queue_num=(2 * j + hi) % 4,
                        )
                    Gs[j] = G

                oT = ostagep.tile([P, gw], f32, tag="oT")

                # dense batches of up to 4 tiles (512-wide moving operand)
                for b0 in range(t0g, t1g, 4):
                    b1_ = min(b0 + 4, t1g)
                    bw = min(b1_ * P, npc) - b0 * P
                    boff = b0 * P - t0g * P
                    aT = densep.tile([P, bw], f16, tag="aT")
                    mT = densep.tile([P, bw], f16, tag="mT")
                    for t in range(b0, b1_):
                        w = min((t + 1) * P, npc) - t * P
                        uses = tile_uses[t]
                        xT = psx.tile([P, P], f32, tag="xT")
                        for ui, (j, gblk, ul) in enumerate(uses):
                            bi = gblk - int(C[g, j])
                            su = int(U0[g, j]) - u0g + ul
                            nc.tensor.matmul(
                                out=xT[:],
                                lhsT=Gs[j][:, bi, :],
                                rhs=S[:, su, :],
                                start=(ui == 0),
                                stop=(ui == len(uses) - 1),
                            )
                        toff = (t - b0) * P
                        fslice = fT[:, t * P - t0g * P : t * P - t0g * P + w]
                        nc.vector.tensor_tensor(
                            out=aT[:, toff : toff + w],
                            in0=xT[:, :w],
                            in1=fslice,
                            op=mybir.AluOpType.add,
                        )
                        nc.vector.tensor_tensor(
                            out=mT[:, toff : toff + w],
                            in0=xT[:, :w],
                            in1=fslice,
                            op=mybir.AluOpType.mult,
                        )
                    out2 = pso.tile([P, bw], f32, tag="out2")
                    nc.tensor.matmul(
                        out=out2[:, :bw], lhsT=w1_t[:], rhs=aT[:, :bw], start=True, stop=False
                    )
                    nc.tensor.matmul(
                        out=out2[:, :bw], lhsT=w2_t[:], rhs=mT[:, :bw], start=False, stop=True
                    )
                    nc.scalar.activation(
                        out=oT[:, boff : boff + bw],
                        in_=out2[:, :bw],
                        func=mybir.ActivationFunctionType.Identity,
                        bias=bias_t[:, :1],
                        scale=1.0,
                    )

                nc.scalar.dma_start(out=outT[:, t0g * P : t0g * P + gw], in_=oT[:, :gw])
    nc.compile()
    return nc


def _run(rows, cols, vals, features, W1, b1, W2, b2, n_nodes, n_cores):
    global _LAST_RESULTS
    from concourse import bass_utils

    npc = n_nodes // n_cores
    features = np.ascontiguousarray(np.asarray(features, dtype=np.float32))
    W1_16 = np.ascontiguousarray(np.asarray(W1, dtype=np.float32).astype(np.float16))
    W2_16 = np.ascontiguousarray(np.asarray(W2, dtype=np.float32).astype(np.float16))
    bsum = np.ascontiguousarray(
        (np.asarray(b1, dtype=np.float32) + np.asarray(b2, dtype=np.float32)).reshape(
            D, 1
        )
    )

    sched, per_core = _prep(rows, cols, vals, n_nodes, n_cores)
    nc = _build_program(n_nodes, sched)

    cc = sched["cc"]
    feat16 = np.ascontiguousarray(features.astype(np.float16))
    feat16_chunks = [
        np.ascontiguousarray(feat16[j * cc : (j + 1) * cc, :]) for j in range(NCHUNKS)
    ]

    in_maps = []
    for c in range(n_cores):
        featT_c = np.ascontiguousarray(features[c * npc : (c + 1) * npc, :].T)
        im = {
            "featT": featT_c,
            "W1": W1_16,
            "W2": W2_16,
            "bsum": bsum,
            "idx16": per_core[c]["idx16"],
            "S16": per_core[c]["S16"],
        }
        for j in range(NCHUNKS):
            im[f"feat16_{j}"] = feat16_chunks[j]
        in_maps.append(im)

    res = bass_utils.run_bass_kernel_spmd(nc, in_maps, core_ids=list(range(n_cores)))
    _LAST_RESULTS = res
    out = np.concatenate([r["outT"].T for r in res.results], axis=0)
    return np.ascontiguousarray(out)


def kernel(rows, cols, vals, features, W1, b1, W2, b2):
    return _run(rows, cols, vals, features, W1, b1, W2, b2, N_NODES, N_CORES)


# revision 53
# speedup vs baseline: 1.0055x; 1.0055x over previous
"""BiGNN message-passing kernel for Trainium2 (8 NeuronCores, Bass/Tile).

Reference computation (N=100000 nodes, E=600000 edges, D=128):
    msgs = vals[:, None] * features[cols]            # gather + scale
    x    = segment_sum(msgs, rows)                   # scatter-add to rows
    out  = (features + x) @ W1 + b1 + (x * features) @ W2 + b2

Sharding: destination nodes (rows) are sharded across the 8 cores, 12500
each; `features` is replicated into every core's HBM, so the per-edge
source gather is core-local (no collectives).

The critical path is GPSIMD descriptor generation for the per-edge
feature gather (dma_gather on 4 SWDGE queues, ~2ns/row serial on the
Pool engine), so the layout minimizes gathered rows and keeps everything
else off that path:

- Edges are packed per (tile-group, feature-chunk) section with NO
  per-tile alignment: each (tile, chunk) run is laid out contiguously,
  runs back to back, sections rounded up to 128 slots. One dma_gather
  per section (32 total vs 52 in the padded-block layout, ~15% fewer
  gathered rows than 128-aligned packing). The last two groups are
  small (8 and 6 tiles) so the post-last-gather epilogue tail is short.
- The segment-sum runs on TensorE as one matmul per (tile, 128-slot
  block) USE: xT[f,d] += G_blk[e,f].T @ S_use[e,d]. Because blocks can
  straddle tile boundaries, S carries one 128-wide one-hot column group
  PER USE (not per block): rows of foreign tiles are zero in that use's
  columns, so the matmul always contracts over the full 128 partitions
  (PE sub-partition operands crash the device, and on-device S builds
  measured ~3x slower on DVE than the ~34MB/core HWDGE stream of
  host-built S, which overlaps the gather-limited critical path).
- Dense epilogue in transposed [feature, node] layout:
  outT = W1.T @ (fT + xT) + W2.T @ (xT * fT) + (b1 + b2); the host
  transposes per-core outputs back.
"""

import numpy as np

P = 128
D = 128
N_NODES = 100000
N_EDGES = 600000
N_CORES = 8
NCHUNKS = 4  # feature-table column chunks (int16 gather index reach)
GROUP_SIZES = [14, 14, 14, 14, 14, 14, 8, 6]  # dest tiles per group; small
# tail groups shrink the post-last-gather epilogue on the critical path

_LAST_RESULTS = None  # BassKernelResults of the most recent run (for test.py)


def _prep(rows, cols, vals, n_nodes, n_cores):
    """Host-side edge reorganization into the shared slot/use schedule.

    Returns (sched, per_core):
      sched: tiles/npc/cc/NG/secblk/C/NB/TOT/NU/U0/tile_uses
      per_core[c]: idx16 [128, TOT/16] int16,
                   dest16/val16 [128, NU] fp16 (per-use masked columns)
    """
    npc = n_nodes // n_cores
    tiles = (npc + P - 1) // P
    cc = n_nodes // NCHUNKS
    assert sum(GROUP_SIZES) == tiles
    NG = len(GROUP_SIZES)
    gb = np.concatenate([[0], np.cumsum(GROUP_SIZES)])  # group tile bounds
    g_of = np.repeat(np.arange(NG), GROUP_SIZES)  # tile -> group
    assert n_nodes % NCHUNKS == 0

    rows = np.asarray(rows, dtype=np.int64)
    cols = np.asarray(cols, dtype=np.int64)
    vals = np.asarray(vals, dtype=np.float32)
    e = rows.shape[0]

    core = rows // npc
    local = rows - core * npc
    t_idx = local // P
    dest = (local - t_idx * P).astype(np.int64)
    j_idx = cols // cc
    lidx = cols - j_idx * cc

    key = (core * tiles + t_idx) * NCHUNKS + j_idx
    order = np.argsort(key, kind="stable")
    cnt = np.bincount(key, minlength=n_cores * tiles * NCHUNKS).reshape(
        n_cores, tiles, NCHUNKS
    )
    L = cnt.max(axis=0).astype(np.int64)  # [tiles, NCHUNKS] shared run lengths

    # sections (group, chunk): contiguous unaligned runs, rounded to 128
    A = np.zeros((tiles, NCHUNKS), dtype=np.int64)  # run offset within section
    secblk = np.zeros((NG, NCHUNKS), dtype=np.int64)
    for g in range(NG):
        t0, t1 = int(gb[g]), int(gb[g + 1])
        for j in range(NCHUNKS):
            off = 0
            for t in range(t0, t1):
                A[t, j] = off
                off += L[t, j]
            secblk[g, j] = max((off + P - 1) // P, 1)
    C = np.zeros((NG, NCHUNKS), dtype=np.int64)  # global block offset
    nb = 0
    for g in range(NG):
        for j in range(NCHUNKS):
            C[g, j] = nb
            nb += secblk[g, j]
    NB = nb
    TOT = NB * P

    # per-edge slot: section base + run offset + rank within (core, t, j)
    starts_flat = np.concatenate([[0], np.cumsum(cnt.reshape(-1))[:-1]])
    rank = np.empty(e, dtype=np.int64)
    rank[order] = np.arange(e) - np.repeat(starts_flat, cnt.reshape(-1))
    slot = C[g_of[t_idx], j_idx] * P + A[t_idx, j_idx] + rank

    # shared slot->tile ownership map (core independent)
    tileof = np.full(TOT, -1, dtype=np.int64)
    for t in range(tiles):
        g = int(g_of[t])
        for j in range(NCHUNKS):
            base = C[g, j] * P + A[t, j]
            tileof[base : base + L[t, j]] = t

    # uses: one matmul (and one S column group) per (tile, touched block)
    sec_uses = {}  # (g, j) -> list of (t, gblk)
    tile_uses = [[] for _ in range(tiles)]
    ul0 = np.zeros((tiles, NCHUNKS), dtype=np.int64)  # first use (in-section)
    blk0 = np.zeros((tiles, NCHUNKS), dtype=np.int64)  # first block of run
    for g in range(NG):
        t0, t1 = int(gb[g]), int(gb[g + 1])
        for j in range(NCHUNKS):
            ulist = []
            for t in range(t0, t1):
                a = A[t, j]
                b = a + L[t, j]
                if b == a:
                    continue
                ul0[t, j] = len(ulist)
                blk0[t, j] = a // P
                for bi in range(a // P, (b + P - 1) // P):
                    ulist.append((t, int(C[g, j]) + bi))
            sec_uses[(g, j)] = ulist
    # exact per-section use-column layout (no padding)
    U0 = np.zeros((NG, NCHUNKS), dtype=np.int64)  # section first column
    nu = 0
    for g in range(NG):
        for j in range(NCHUNKS):
            U0[g, j] = nu
            nu += len(sec_uses[(g, j)])
    NU = nu
    for g in range(NG):
        for j in range(NCHUNKS):
            for ul, (t, gblk) in enumerate(sec_uses[(g, j)]):
                tile_uses[t].append((j, gblk, ul))
    # reorder each tile's uses to chunk-major (j asc) for the psum chain
    for t in range(tiles):
        tile_uses[t].sort(key=lambda u: (u[0], u[1]))

    # per-edge use column (global)
    u_edge = (
        U0[g_of[t_idx], j_idx]
        + ul0[t_idx, j_idx]
        + (slot // P - (C[g_of[t_idx], j_idx] + blk0[t_idx, j_idx]))
    )

    per_core = []
    for c in range(n_cores):
        m = core == c
        s = slot[m]
        idx_flat = np.zeros(TOT, dtype=np.int16)
        idx_flat[s] = lidx[m].astype(np.int16)
        idx16 = np.tile(np.ascontiguousarray(idx_flat.reshape(-1, 16).T), (8, 1))
        # host-built per-use one-hot S: S16[p, u*128 + d] = val
        S16 = np.zeros((P, NU * P), dtype=np.float16)
        S16[s % P, u_edge[m] * P + dest[m]] = vals[m].astype(np.float16)
        per_core.append(
            {
                "idx16": np.ascontiguousarray(idx16),
                "S16": S16,
            }
        )

    sched = {
        "tiles": tiles,
        "npc": npc,
        "cc": cc,
        "NG": NG,
        "secblk": secblk,
        "C": C,
        "NB": NB,
        "TOT": TOT,
        "NU": NU,
        "U0": U0,
        "gb": gb,
        "tile_uses": tile_uses,
    }
    return sched, per_core


def _build_program(n_nodes, sched):
    import concourse.bacc as bacc
    import concourse.mybir as mybir
    import concourse.tile as tile

    f32 = mybir.dt.float32
    f16 = mybir.dt.float16
    i16 = mybir.dt.int16

    npc = sched["npc"]
    cc = sched["cc"]
    NG = sched["NG"]
    secblk = sched["secblk"]
    C = sched["C"]
    NB = sched["NB"]
    TOT = sched["TOT"]
    NU = sched["NU"]
    U0 = sched["U0"]
    gb = sched["gb"]
    tiles = sched["tiles"]
    tile_uses = sched["tile_uses"]

    nc = bacc.Bacc(num_swdge_queues=4)
    feat16 = [
        nc.dram_tensor(f"feat16_{j}", [cc, D], f16, kind="ExternalInput")
        for j in range(NCHUNKS)
    ]
    featT = nc.dram_tensor("featT", [D, npc], f32, kind="ExternalInput")
    w1 = nc.dram_tensor("W1", [D, D], f16, kind="ExternalInput")
    w2 = nc.dram_tensor("W2", [D, D], f16, kind="ExternalInput")
    bsum = nc.dram_tensor("bsum", [D, 1], f32, kind="ExternalInput")
    idx16 = nc.dram_tensor("idx16", [P, TOT // 16], i16, kind="ExternalInput")
    s16 = nc.dram_tensor("S16", [P, NU * P], f16, kind="ExternalInput")
    outT = nc.dram_tensor("outT", [D, npc], f32, kind="ExternalOutput")

    with tile.TileContext(nc) as tc:
        with (
            tc.tile_pool(name="const", bufs=1) as constp,
            tc.tile_pool(name="idxp", bufs=2) as idxp,
            tc.tile_pool(name="gpool", bufs=2) as gpool,
            tc.tile_pool(name="spool", bufs=2) as spool,
            tc.tile_pool(name="ftpool", bufs=2) as ftpool,
            tc.tile_pool(name="dense", bufs=3) as densep,
            tc.tile_pool(name="ostage", bufs=2) as ostagep,
            tc.tile_pool(name="psx", bufs=6, space="PSUM") as psx,
            tc.tile_pool(name="pso", bufs=2, space="PSUM") as pso,
        ):
            # consts ride the scalar (ACT) ring so the group-0 gather index
            # load is first in the sync HWDGE queue
            w1_t = constp.tile([P, P], f16)
            nc.scalar.dma_start(out=w1_t[:], in_=w1[:, :])
            w2_t = constp.tile([P, P], f16)
            nc.scalar.dma_start(out=w2_t[:], in_=w2[:, :])
            bias_t = constp.tile([P, 1], f32)
            nc.scalar.dma_start(out=bias_t[:], in_=bsum[:, :])

            for g in range(NG):
                t0g = int(gb[g])
                t1g = int(gb[g + 1])
                gw = min(t1g * P, npc) - t0g * P
                b0g = int(C[g, 0])
                b1g = int(C[g, NCHUNKS - 1] + secblk[g, NCHUNKS - 1])
                gnb = b1g - b0g
                u0g = int(U0[g, 0])
                u1g = int(U0[g + 1, 0]) if g + 1 < NG else NU
                gnu = u1g - u0g

                # per-group streams: gather indices first (gathers wait on it)
                ix = idxp.tile([P, gnb * 8], i16, tag="ix")
                nc.sync.dma_start(out=ix[:], in_=idx16[:, b0g * 8 : b1g * 8])
                # host-built per-use one-hot S for the whole group
                S = spool.tile([P, gnu, P], f16, tag="S")
                nc.sync.dma_start(out=S[:], in_=s16[:, u0g * P : (u0g + gnu) * P])
                fT = ftpool.tile([P, gw], f32, tag="fT")
                nc.scalar.dma_start(out=fT[:], in_=featT[:, t0g * P : t0g * P + gw])

                Gs = {}
                for j in range(NCHUNKS):
                    nblk = int(secblk[g, j])
                    di = int(C[g, j]) - b0g
                    G = gpool.tile([P, nblk, P], f16, tag=f"G{j}")
                    n_idx = nblk * P
                    nc.gpsimd.dma_gather(
                        G[:],
                        feat16[j][:, :],
                        ix[:, di * 8 : di * 8 + n_idx // 16],
                        n_idx,
                        n_idx,
                        D,
                        single_packet=False,
                        queue_num=j,
                    )
                    Gs[j] = G

                oT = ostagep.tile([P, gw], f32, tag="oT")

                # dense batches of up to 4 tiles (512-wide moving operand)
                for b0 in range(t0g, t1g, 4):
                    b1_ = min(b0 + 4, t1g)
                    bw = min(b1_ * P, npc) - b0 * P
                    boff = b0 * P - t0g * P
                    aT = densep.tile([P, bw], f16, tag="aT")
                    mT = densep.tile([P, bw], f16, tag="mT")
                    for t in range(b0, b1_):
                        w = min((t + 1) * P, npc) - t * P
                        uses = tile_uses[t]
                        xT = psx.tile([P, P], f32, tag="xT")
                        for ui, (j, gblk, ul) in enumerate(uses):
                            bi = gblk - int(C[g, j])
                            su = int(U0[g, j]) - u0g + ul
                            nc.tensor.matmul(
                                out=xT[:],
                                lhsT=Gs[j][:, bi, :],
                                rhs=S[:, su, :],
                                start=(ui == 0),
                                stop=(ui == len(uses) - 1),
                            )
                        toff = (t - b0) * P
                        fslice = fT[:, t * P - t0g * P : t * P - t0g * P + w]
                        nc.vector.tensor_tensor(
                            out=aT[:, toff : toff + w],
                            in0=xT[:, :w],
                            in1=fslice,
                            op=mybir.AluOpType.add,
                        )
                        nc.vector.tensor_tensor(
                            out=mT[:, toff : toff + w],
                            in0=xT[:, :w],
                            in1=fslice,
                            op=mybir.AluOpType.mult,
                        )
                    out2 = pso.tile([P, bw], f32, tag="out2")
                    nc.tensor.matmul(
                        out=out2[:, :bw], lhsT=w1_t[:], rhs=aT[:, :bw], start=True, stop=False
                    )
                    nc.tensor.matmul(
                        out=out2[:, :bw], lhsT=w2_t[:], rhs=mT[:, :bw], start=False, stop=True
                    )
                    nc.scalar.activation(
                        out=oT[:, boff : boff + bw],
                        in_=out2[:, :bw],
                        func=mybir.ActivationFunctionType.Identity,
                        bias=bias_t[:, :1],
                        scale=1.0,
                    )

                nc.scalar.dma_start(out=outT[:, t0g * P : t0g * P + gw], in_=oT[:, :gw])
    nc.compile()
    return nc


def _run(rows, cols, vals, features, W1, b1, W2, b2, n_nodes, n_cores):
    global _LAST_RESULTS
    from concourse import bass_utils

    npc = n_nodes // n_cores
    features = np.ascontiguousarray(np.asarray(features, dtype=np.float32))
    W1_16 = np.ascontiguousarray(np.asarray(W1, dtype=np.float32).astype(np.float16))
    W2_16 = np.ascontiguousarray(np.asarray(W2, dtype=np.float32).astype(np.float16))
    bsum = np.ascontiguousarray(
        (np.asarray(b1, dtype=np.float32) + np.asarray(b2, dtype=np.float32)).reshape(
            D, 1
        )
    )

    sched, per_core = _prep(rows, cols, vals, n_nodes, n_cores)
    nc = _build_program(n_nodes, sched)

    cc = sched["cc"]
    feat16 = np.ascontiguousarray(features.astype(np.float16))
    feat16_chunks = [
        np.ascontiguousarray(feat16[j * cc : (j + 1) * cc, :]) for j in range(NCHUNKS)
    ]

    in_maps = []
    for c in range(n_cores):
        featT_c = np.ascontiguousarray(features[c * npc : (c + 1) * npc, :].T)
        im = {
            "featT": featT_c,
            "W1": W1_16,
            "W2": W2_16,
            "bsum": bsum,
            "idx16": per_core[c]["idx16"],
            "S16": per_core[c]["S16"],
        }
        for j in range(NCHUNKS):
            im[f"feat16_{j}"] = feat16_chunks[j]
        in_maps.append(im)

    res = bass_utils.run_bass_kernel_spmd(nc, in_maps, core_ids=list(range(n_cores)))
    _LAST_RESULTS = res
    out = np.concatenate([r["outT"].T for r in res.results], axis=0)
    return np.ascontiguousarray(out)


def kernel(rows, cols, vals, features, W1, b1, W2, b2):
    return _run(rows, cols, vals, features, W1, b1, W2, b2, N_NODES, N_CORES)
